# revision 17
# baseline (speedup 1.0000x reference)
"""BayesianKAN ECG kernel for Trainium2 (8 NeuronCores, data-parallel batch shard).

Pipeline per 1024-row pair of 512-row blocks:
  x[128,2,1000] --gp add + DVE reduce--> pooled[128,8,100] --bn_stats-->
  batched Newton rsqrt --> normalize (fp32) --PE transpose--> xnT[100,1024]
  b1[100,12,1024] bf16 (reduced 12-Gaussian basis, host-refit weights):
    3 ACT anchor slices (Square+Exp) + ru/rd exps + 2 batched DVE ring muls
  --12 PE matmuls bf16--> h[64,1024] --Tanh--> hb bf16
  --8 DMA-transposes--> layout A --stats+Newton+normalize (fp32)--> hn
  --8 PE transposes--> b2s[128,1024] fp32 (DMA replicate)
  b2[128,6,1024] bf16: 1 ACT anchor chunk + ru2/rd2 + 4 DVE ring muls
  --12 PE matmuls--> out[5,1024] fp32 --> DRAM
Notes:
  * mean/std scale-invariant so pooling /10 skipped.
  * ddof=1 via biased-var scale n/(n-1) before Newton rsqrt.
  * the 16-Gaussian basis is least-squares projected onto 12 wider-spaced
    Gaussians on the host (weighted by the N(0,1) input density); residual
    ~1e-3 relative.
  * basis slices away from anchors use B_n = B_a * r^(n-a) with
    r = exp(2*A*delta*x); the constant exp(-A*(c_n^2-c_a^2)) is folded into
    the matmul weights on the host.
  * PSUM->SBUF copies ride the Scalar engine (AF.Copy) to offload DVE.
"""

import os
import sys
from contextlib import ExitStack

import numpy as np
import ml_dtypes

sys.path.insert(0, "/opt/trn_rl_repo")

import concourse.bass as bass
import concourse.tile as tile
from concourse import mybir
from concourse.bass_utils import run_bass_kernel_spmd

from concourse import bass2jax as _b2j

_orig_hook = _b2j.neuronx_cc_hook


def _dbg_hook(*a, **k):
    try:
        return _orig_hook(*a, **k)
    except BaseException:
        import traceback
        with open("/tmp/hook_err.txt", "w") as f:
            traceback.print_exc(file=f)
        raise


_b2j.neuronx_cc_hook = _dbg_hook


MAX_WAITS = 1


def _split_sync_waits(nc, limit=MAX_WAITS):
    """Walrus in this env rejects instructions with more than ~2 sync waits.

    Move excess waits onto same-engine NOPs inserted right before the
    offending instruction (in-order engines make this equivalent).
    """
    n_split = 0
    for block in nc.main_func.blocks:
        new_insts = []
        for inst in block.instructions:
            si = inst.sync_info
            waits = list(si.on_wait) if si is not None else []
            if len(waits) > limit:
                extra, keep = waits[:-limit], waits[-limit:]
                for k in range(0, len(extra), limit):
                    nop = mybir.InstNoOp(
                        name=f"{inst.name}-ws{k}",
                        sync_info=mybir.SyncInfo(
                            on_wait=extra[k : k + limit], on_update=[]
                        ),
                        bass_nofuse=True,
                        engine=inst.engine,
                    )
                    nc.register_instruction(nop, overwrite=True)
                    new_insts.append(nop)
                    n_split += 1
                si.on_wait = keep
                inst.sync_info = si
            new_insts.append(inst)
        block.instructions[:] = new_insts
    return n_split


BATCH = 65536
SEQ = 1000
IN_DIM = 100
POOLW = 10
HID = 64
OUT_DIM = 5
NB = 16
NCORES = 8
ROWS = BATCH // NCORES          # 8192 rows per core
TILE_P = 128
BLK = 512                       # rows per block
NSUB = BLK // TILE_P            # 4 sub-tiles per block
NBLK = ROWS // BLK              # 16 blocks per core
PBLK = 2 * BLK                  # 1024 rows per pair

NB1 = 12                        # reduced basis size
NCH2 = NB1 // 2                 # b2 chunks
B1_ANCH = (1, 5, 9)             # direct b1 slices
B2_ANCH = 2                     # direct b2 chunk (n = 4, 5)

LAST_RESULTS = None


def _ensure_ntff_hook():
    """Synthesize antenv.axon_hooks (absent in this image) so trace=True works."""
    import types

    if "antenv.axon_hooks" in sys.modules:
        return
    mod = types.ModuleType("antenv.axon_hooks")
    mod._hook = None

    def set_axon_ntff_profile_hook(h):
        mod._hook = h

    def get_axon_ntff_profile_hook():
        return mod._hook

    mod.set_axon_ntff_profile_hook = set_axon_ntff_profile_hook
    mod.get_axon_ntff_profile_hook = get_axon_ntff_profile_hook
    sys.modules["antenv.axon_hooks"] = mod
    import antenv

    antenv.axon_hooks = mod
    try:
        from trn_agent_boot.trn_boot import _ntff_profile_via_ctypes

        hook = _ntff_profile_via_ctypes("/opt/axon/libaxon_pjrt.so")
        if hook is not None:
            set_axon_ntff_profile_hook(hook)
    except Exception as e:
        print("ntff hook setup failed:", e)


RBF_A = np.float32(0.5 / 0.36)  # exp(-A * d^2)
MAGIC = 0x5F3759DF

F32 = mybir.dt.float32
BF16 = mybir.dt.bfloat16
I32 = mybir.dt.int32
AF = mybir.ActivationFunctionType
ALU = mybir.AluOpType
AX = mybir.AxisListType


def _newton_rsqrt(nc, pool, var_ap, out_ap, n_par, n_free, ddof_scale):
    """out = rsqrt(var * ddof_scale) elementwise on a [n_par, n_free] AP.

    var_ap is strided fp32; Newton w/ fast-inverse-sqrt seed, 2 iterations.
    """
    v = pool.tile([n_par, n_free], F32, tag="nw_v")
    y = pool.tile([n_par, n_free], F32, tag="nw_y")
    t = pool.tile([n_par, n_free], F32, tag="nw_t")
    nc.vector.tensor_scalar(v, var_ap, float(ddof_scale), None, ALU.mult)
    nc.vector.tensor_scalar(
        y.bitcast(I32), v.bitcast(I32), 1, None, ALU.logical_shift_right
    )
    nc.vector.tensor_scalar(
        y.bitcast(I32), y.bitcast(I32), -1, MAGIC, ALU.mult, ALU.add
    )
    nc.vector.tensor_mul(t, y, y)
    nc.vector.tensor_mul(t, t, v)
    nc.vector.tensor_scalar(t, t, -0.5, 1.5, ALU.mult, ALU.add)
    nc.vector.tensor_mul(out_ap, y, t)


def build_bass(cen12, nblk=NBLK):
    rows = nblk * BLK
    npairs = nblk // 2
    assert nblk % 2 == 0
    nc = bass.Bass()

    x_in = nc.declare_dram_parameter("x", [rows, SEQ], F32, isOutput=False)
    w1_in = nc.declare_dram_parameter("w1", [IN_DIM, NB1 * HID], BF16, isOutput=False)
    w2_in = nc.declare_dram_parameter(
        "w2", [TILE_P, NCH2 * OUT_DIM], BF16, isOutput=False
    )
    cb1_in = nc.declare_dram_parameter("cb1", [IN_DIM, 3], F32, isOutput=False)
    cb2a_in = nc.declare_dram_parameter("cb2a", [TILE_P, 1], F32, isOutput=False)
    ident_in = nc.declare_dram_parameter("ident", [TILE_P, TILE_P], F32, isOutput=False)
    identb_in = nc.declare_dram_parameter("identb", [HID, HID], BF16, isOutput=False)
    out_ext = nc.declare_dram_parameter("out", [OUT_DIM, rows], F32, isOutput=True)

    cvals = [float(c) for c in cen12]
    delta = cvals[1] - cvals[0]
    A = float(RBF_A)
    RU_S = 2.0 * A * delta          # ru = exp(+RU_S * x), rd = exp(-RU_S * x)
    RU2_S = 4.0 * A * delta         # chunk step is 2*delta

    with ExitStack() as ctx:
        tc = ctx.enter_context(tile.TileContext(nc))
        singles = ctx.enter_context(tc.tile_pool(name="singles", bufs=1))
        xin_p = ctx.enter_context(tc.tile_pool(name="xin", bufs=6))
        xh_p = ctx.enter_context(tc.tile_pool(name="xh", bufs=4))
        xp_p = ctx.enter_context(tc.tile_pool(name="xp", bufs=3))
        small_p = ctx.enter_context(tc.tile_pool(name="small", bufs=4))
        nw_p = ctx.enter_context(tc.tile_pool(name="newton", bufs=2))
        xn_p = ctx.enter_context(tc.tile_pool(name="xn", bufs=2))
        xnt_p = ctx.enter_context(tc.tile_pool(name="xnt", bufs=2))
        b1sq_p = ctx.enter_context(tc.tile_pool(name="b1sq", bufs=1))
        b1_p = ctx.enter_context(tc.tile_pool(name="b1", bufs=2))
        rdru_p = ctx.enter_context(tc.tile_pool(name="rdru", bufs=2))
        hb_p = ctx.enter_context(tc.tile_pool(name="hb", bufs=2))
        ha_p = ctx.enter_context(tc.tile_pool(name="ha", bufs=2))
        hn_p = ctx.enter_context(tc.tile_pool(name="hn", bufs=2))
        b2s_p = ctx.enter_context(tc.tile_pool(name="b2s", bufs=2))
        b2sq_p = ctx.enter_context(tc.tile_pool(name="b2sq", bufs=1))
        rdru2_p = ctx.enter_context(tc.tile_pool(name="rdru2", bufs=1))
        b2_p = ctx.enter_context(tc.tile_pool(name="b2", bufs=2))
        outs_p = ctx.enter_context(tc.tile_pool(name="outs", bufs=3))
        ps_t = ctx.enter_context(tc.tile_pool(name="ps_t", bufs=2, space="PSUM"))
        ps_h = ctx.enter_context(tc.tile_pool(name="ps_h", bufs=1, space="PSUM"))
        ps_x2 = ctx.enter_context(tc.tile_pool(name="ps_x2", bufs=1, space="PSUM"))
        ps_o = ctx.enter_context(tc.tile_pool(name="ps_o", bufs=1, space="PSUM"))
        ps_a = ctx.enter_context(tc.tile_pool(name="ps_a", bufs=1, space="PSUM"))

        ident = singles.tile([TILE_P, TILE_P], F32)
        nc.sync.dma_start(out=ident, in_=ident_in[:, :])
        identb = singles.tile([HID, HID], BF16)
        nc.sync.dma_start(out=identb, in_=identb_in[:, :])
        w1 = singles.tile([IN_DIM, NB1, HID], BF16)
        nc.sync.dma_start(out=w1, in_=w1_in[:, :].rearrange("i (n o) -> i n o", n=NB1))
        w2 = singles.tile([TILE_P, NCH2, OUT_DIM], BF16)
        nc.sync.dma_start(out=w2, in_=w2_in[:, :].rearrange("p (c o) -> p c o", c=NCH2))
        cb1 = singles.tile([IN_DIM, 3], F32)
        nc.sync.dma_start(out=cb1, in_=cb1_in[:, :])
        cb2a = singles.tile([TILE_P, 1], F32)
        nc.sync.dma_start(out=cb2a, in_=cb2a_in[:, :])

        # rows index = ((b*2 + s2)*2 + k)*128 + p
        x_t = x_in[:, :].rearrange(
            "(nb s2 k p) s -> nb s2 p k s", nb=nblk, s2=2, k=2
        )

        pairs = {}

        def stage_a(b):
            """DMA + pool + stats for block b into the pair's tiles."""
            pr = b // 2
            if b % 2 == 0:
                pairs[pr] = {
                    "xp": xp_p.tile([TILE_P, 8, IN_DIM], F32, tag="xp", name="xp"),
                    "mv1": small_p.tile([TILE_P, 2, 8], F32, tag="mv1", name="mv1"),
                }
            d = pairs[pr]
            base = (b % 2) * NSUB
            for s2 in range(2):
                x2 = xin_p.tile([TILE_P, 2, SEQ], F32, tag="x")
                nc.sync.dma_start(out=x2, in_=x_t[b, s2])
                xf = x2.rearrange("p k (i w) -> p k i w", w=POOLW)
                xh = xh_p.tile([TILE_P, 2, IN_DIM, POOLW // 2], F32, tag="xh")
                nc.gpsimd.tensor_tensor(
                    xh, xf[:, :, :, 0 : POOLW // 2], xf[:, :, :, POOLW // 2 :],
                    ALU.add,
                )
                sg = base + 2 * s2
                nc.vector.tensor_reduce(
                    d["xp"][:, sg : sg + 2, :], xh, AX.X, ALU.add
                )
                for k in range(2):
                    st6 = small_p.tile([TILE_P, 6], F32, tag="st6")
                    nc.vector.bn_stats(st6, d["xp"][:, sg + k, :])
                    nc.vector.bn_aggr(d["mv1"][:, :, sg + k : sg + k + 1], st6)

        def stage_t(pr):
            """Batched Newton + normalize + PE transpose -> xnt fp32."""
            d = pairs[pr]
            r1 = small_p.tile([TILE_P, 8], F32, tag="r1")
            _newton_rsqrt(
                nc, nw_p, d["mv1"][:, 1, :], r1, TILE_P, 8, IN_DIM / (IN_DIM - 1)
            )
            xn = xn_p.tile([TILE_P, 8, IN_DIM], F32, tag="xn")
            pss = []
            for half in range(2):
                xnt_ps = ps_t.tile([IN_DIM, BLK], F32, tag="xnt_ps")
                for s in range(NSUB):
                    sg = half * NSUB + s
                    nc.vector.tensor_scalar(
                        xn[:, sg, :], d["xp"][:, sg, :],
                        d["mv1"][:, 0, sg : sg + 1], r1[:, sg : sg + 1],
                        ALU.subtract, ALU.mult,
                    )
                    nc.tensor.transpose(
                        xnt_ps[:, s * TILE_P : (s + 1) * TILE_P],
                        xn[:, sg, :], ident,
                    )
                pss.append(xnt_ps)
            del pairs[pr]
            return pss

        def stage_l1_basis(pr, pss):
            """b1 basis: ACT anchors/ratios + DVE ring muls (no matmuls)."""
            b1sq = b1sq_p.tile([IN_DIM, 3, PBLK], F32, tag="b1sq")
            b1 = b1_p.tile([IN_DIM, NB1, PBLK], BF16, tag="b1")
            rdru = rdru_p.tile([IN_DIM, 2, PBLK], BF16, tag="rdru")
            for half in range(2):
                sl = slice(half * BLK, (half + 1) * BLK)
                nc.scalar.activation(rdru[:, 0, sl], pss[half], AF.Exp, scale=-RU_S)
                nc.scalar.activation(rdru[:, 1, sl], pss[half], AF.Exp, scale=RU_S)
                for j, n in enumerate(B1_ANCH):
                    nc.scalar.activation(
                        b1sq[:, j, sl], pss[half], AF.Square,
                        bias=cb1[:, j : j + 1],
                    )
            b1g4 = b1.rearrange("p (g a) k -> p g a k", a=4)
            nc.scalar.activation(
                b1g4[:, :, 1, :], b1sq, AF.Exp, scale=float(-RBF_A)
            )
            # ring1: slices 4g (rd) and 4g+2 (ru) from anchor 4g+1, one op
            b1r = b1.rearrange("p (g j b) k -> p g j b k", j=2, b=2)
            dst = b1r[:, :, :, 0, :]
            src = b1g4[:, :, 1, :].unsqueeze(2).broadcast_to((IN_DIM, 3, 2, PBLK))
            mlt = rdru.unsqueeze(1).broadcast_to((IN_DIM, 3, 2, PBLK))
            nc.vector.tensor_tensor(dst, src, mlt, ALU.mult)
            # ring2: slices {3,7,11} = {2,6,10} * ru, one op
            nc.vector.tensor_tensor(
                b1g4[:, :, 3, :], b1g4[:, :, 2, :],
                rdru[:, 1, :].unsqueeze(1).broadcast_to((IN_DIM, 3, PBLK)),
                ALU.mult,
            )
            return b1

        def stage_l1_mm(pr, b1):
            """mm1 + tanh -> hb bf16."""
            h_ps = ps_h.tile([HID, PBLK], F32, tag="h_ps")
            mm_order = list(B1_ANCH) + [0, 2, 4, 6, 8, 10, 3, 7, 11]
            for mi, n in enumerate(mm_order):
                for hf in range(2):
                    nc.tensor.matmul(
                        h_ps[:, hf * BLK : (hf + 1) * BLK],
                        w1[:, n, :],
                        b1[:, n, hf * BLK : (hf + 1) * BLK],
                        start=(mi == 0), stop=(mi == len(mm_order) - 1),
                    )
            hb = hb_p.tile([HID, PBLK], BF16, tag="hb")
            nc.scalar.activation(hb, h_ps, AF.Tanh)
            return hb

        def stage_l2_pre_a(pr, hb):
            """norm2 front: PE transpose to layout A (PSUM), stats, Newton,
            normalize (reads PSUM directly)."""
            ha_ps = ps_a.tile([TILE_P, 8 * HID], BF16, tag="ha_ps")
            for c in range(8):
                nc.tensor.transpose(
                    ha_ps[:, c * HID : (c + 1) * HID],
                    hb[:, c * TILE_P : (c + 1) * TILE_P], identb,
                )
            mv2 = small_p.tile([TILE_P, 2, 8], F32, tag="mv2")
            r2 = small_p.tile([TILE_P, 8], F32, tag="r2")
            for c in range(8):
                st6b = small_p.tile([TILE_P, 6], F32, tag="st6b")
                nc.vector.bn_stats(st6b, ha_ps[:, c * HID : (c + 1) * HID])
                nc.vector.bn_aggr(mv2[:, :, c : c + 1], st6b)
            _newton_rsqrt(
                nc, nw_p, mv2[:, 1, :], r2, TILE_P, 8, HID / (HID - 1)
            )
            hn = hn_p.tile([TILE_P, 8, HID], F32, tag="hn")
            for c in range(8):
                nc.vector.tensor_scalar(
                    hn[:, c, :], ha_ps[:, c * HID : (c + 1) * HID],
                    mv2[:, 0, c : c + 1], r2[:, c : c + 1],
                    ALU.subtract, ALU.mult,
                )
            return hn

        def stage_l2_pre_b(pr, hn):
            """norm2 back: PE transpose to layout B, replicate -> b2s fp32."""
            xn2_ps = ps_x2.tile([HID, PBLK], F32, tag="xn2_ps")
            for c in range(8):
                nc.tensor.transpose(
                    xn2_ps[:, c * TILE_P : (c + 1) * TILE_P], hn[:, c, :], ident
                )
            b2s = b2s_p.tile([TILE_P, PBLK], F32, tag="b2s")
            nc.scalar.activation(b2s[:HID, :], xn2_ps, AF.Copy)
            nc.scalar.dma_start(out=b2s[HID:, :], in_=b2s[:HID, :])
            return b2s

        def stage_l2_basis(pr, b2s):
            """b2 basis: ACT anchor chunk + ratios + DVE ring muls."""
            b2sq = b2sq_p.tile([TILE_P, PBLK], F32, tag="b2sq")
            b2 = b2_p.tile([TILE_P, NCH2, PBLK], BF16, tag="b2")
            rdru2 = rdru2_p.tile([TILE_P, 2, PBLK], BF16, tag="rdru2")
            nc.scalar.activation(rdru2[:, 0, :], b2s, AF.Exp, scale=-RU2_S)
            nc.scalar.activation(rdru2[:, 1, :], b2s, AF.Exp, scale=RU2_S)
            nc.scalar.activation(b2sq, b2s, AF.Square, bias=cb2a[:, 0:1])
            nc.scalar.activation(
                b2[:, B2_ANCH, :], b2sq, AF.Exp, scale=float(-RBF_A)
            )
            # ring1: chunks {1,3} from anchor 2
            b2r2 = b2.rearrange("p (a b) k -> p a b k", b=2)
            dst1 = b2r2[:, 0:2, 1, :]
            src1 = b2[:, B2_ANCH, :].unsqueeze(1).broadcast_to((TILE_P, 2, PBLK))
            nc.vector.tensor_tensor(dst1, src1, rdru2, ALU.mult)
            # chunk 0 from 1 (rd2); chunk 4 from 3 (ru2); chunk 5 from 4 (ru2)
            nc.gpsimd.tensor_tensor(
                b2[:, 0, :], b2[:, 1, :], rdru2[:, 0, :], ALU.mult
            )
            nc.vector.tensor_tensor(
                b2[:, 4, :], b2[:, 3, :], rdru2[:, 1, :], ALU.mult
            )
            nc.vector.tensor_tensor(
                b2[:, 5, :], b2[:, 4, :], rdru2[:, 1, :], ALU.mult
            )
            return b2

        def stage_l2_mm(pr, b2):
            """mm2 + out copies + out DMA."""
            mm2_order = [2, 1, 3, 0, 4, 5]
            for hf in range(2):
                o_ps = ps_o.tile([OUT_DIM, BLK], F32, tag="o_ps")
                for ci, c in enumerate(mm2_order):
                    nc.tensor.matmul(
                        o_ps,
                        w2[:, c, :],
                        b2[:, c, hf * BLK : (hf + 1) * BLK],
                        start=(ci == 0), stop=(ci == len(mm2_order) - 1),
                    )
                out_sb = outs_p.tile([OUT_DIM, BLK], F32, tag="out_sb")
                nc.scalar.activation(out_sb, o_ps, AF.Copy)
                bb = 2 * pr + hf
                nc.scalar.dma_start(
                    out=out_ext[:, bb * BLK : (bb + 1) * BLK], in_=out_sb
                )

        # warm the ACT function table + PE HAM while the first x DMAs fly
        warm = singles.tile([IN_DIM, 8], BF16)
        nc.scalar.activation(warm, cb1[:, 0:1].broadcast_to((IN_DIM, 8)), AF.Exp)
        wps = ps_a.tile([TILE_P, 8 * HID], BF16, tag="ha_ps")
        for _ in range(24):
            nc.tensor.transpose(wps[:HID, :HID], identb, identb)

        # software pipeline (lagged): iteration p emits
        #   T(p), L1basis(p), L2pre_a(p-1), L1mm(p), L2pre_b(p-1),
        #   L2basis(p-1), L2mm(p-2), stage_a(pair p+1 blocks)
        # so each engine FIFO always holds ready work from several pairs.
        stage_a(0)
        stage_a(1)
        hbs, hns, b2ss, b2sb = {}, {}, {}, {}
        for p in range(npairs + 2):
            if p < npairs:
                pss = stage_t(p)
                b1 = stage_l1_basis(p, pss)
            if p - 1 in hbs:
                hns[p - 1] = stage_l2_pre_a(p - 1, hbs.pop(p - 1))
            if p < npairs:
                hbs[p] = stage_l1_mm(p, b1)
            if p - 1 in hns:
                b2ss[p - 1] = stage_l2_pre_b(p - 1, hns.pop(p - 1))
            if p - 1 in b2ss:
                b2sb[p - 1] = stage_l2_basis(p - 1, b2ss.pop(p - 1))
            if p - 2 in b2sb:
                stage_l2_mm(p - 2, b2sb.pop(p - 2))
            for b in (2 * p + 2, 2 * p + 3):
                if b < nblk:
                    stage_a(b)

    _split_sync_waits(nc)
    return nc


def _reduce_basis_matrix(cen16, cen12):
    """Least-squares projection of the 16-Gaussian basis onto 12 Gaussians."""
    A = float(RBF_A)
    xg = np.linspace(-8, 8, 4001)
    wd = np.sqrt(np.exp(-0.5 * xg**2) + 1e-4)
    Phi = np.exp(-A * (xg[:, None] - cen16[None, :]) ** 2)
    Psi = np.exp(-A * (xg[:, None] - cen12[None, :]) ** 2)
    M, *_ = np.linalg.lstsq(Psi * wd[:, None], Phi * wd[:, None], rcond=None)
    return M  # [12, 16]


def _host_consts(c1_mu, c2_mu, centers):
    A = float(RBF_A)
    cen16 = np.asarray(centers, np.float64)
    cen12 = np.linspace(cen16[0], cen16[-1], NB1)
    M = _reduce_basis_matrix(cen16, cen12)
    c1m = np.tensordot(c1_mu.astype(np.float64), M, axes=([2], [1]))  # [o,i,12]
    c2m = np.tensordot(c2_mu.astype(np.float64), M, axes=([2], [1]))  # [o,h,12]

    # b1 chain anchor: every slice chains to 4*(n//4)+1
    w1f = np.transpose(c1m, (1, 2, 0)).copy()  # [i, n, o]
    for n in range(NB1):
        a = 4 * (n // 4) + 1
        if a != n:
            w1f[:, n, :] *= np.exp(-A * (cen12[n] ** 2 - cen12[a] ** 2))
    w1 = w1f.reshape(IN_DIM, NB1 * HID).astype(ml_dtypes.bfloat16)

    # b2: partition p of chunk c holds i = p % 64, n = 2c + p // 64
    w2 = np.zeros((TILE_P, NCH2, OUT_DIM), np.float64)
    cb2a = np.zeros((TILE_P, 1), np.float32)
    for p in range(TILE_P):
        i = p % HID
        na = 2 * B2_ANCH + p // HID
        cb2a[p, 0] = -cen12[na]
        for c in range(NCH2):
            n = 2 * c + p // HID
            k = np.exp(-A * (cen12[n] ** 2 - cen12[na] ** 2)) if c != B2_ANCH else 1.0
            w2[p, c, :] = c2m[:, i, n] * k
    w2 = w2.reshape(TILE_P, NCH2 * OUT_DIM).astype(ml_dtypes.bfloat16)
    cb1 = np.tile(
        -cen12[np.array(B1_ANCH)].astype(np.float32)[None, :], (IN_DIM, 1)
    ).astype(np.float32)
    ident = np.eye(TILE_P, dtype=np.float32)
    identb = np.eye(HID, dtype=ml_dtypes.bfloat16)
    return w1, w2, cb1, cb2a, ident, identb, cen12


def kernel(x, c1_mu, c2_mu, centers):
    x = np.asarray(x, np.float32)
    batch = x.shape[0]
    rows = batch // NCORES
    nblk = rows // BLK
    assert rows % BLK == 0 and nblk % 2 == 0
    c1_mu = np.asarray(c1_mu, np.float32)
    c2_mu = np.asarray(c2_mu, np.float32)
    centers = np.asarray(centers, np.float32)

    w1, w2, cb1, cb2a, ident, identb, cen12 = _host_consts(c1_mu, c2_mu, centers)
    nc = build_bass(cen12, nblk)

    in_maps = []
    for i in range(NCORES):
        in_maps.append(
            {
                "x": np.ascontiguousarray(x[i * rows : (i + 1) * rows]),
                "w1": w1,
                "w2": w2,
                "cb1": cb1,
                "cb2a": cb2a,
                "ident": ident,
                "identb": identb,
            }
        )
    trace = bool(int(os.environ.get("BASS_KERNEL_TRACE", "0")))
    if trace:
        sys.path.insert(0, "/root/.axon_site")
        _ensure_ntff_hook()
    res = run_bass_kernel_spmd(
        nc, in_maps, list(range(NCORES)), trace=trace
    )
    global LAST_RESULTS
    LAST_RESULTS = res
    out = np.empty((batch, OUT_DIM), np.float32)
    for i in range(NCORES):
        out[i * rows : (i + 1) * rows] = res.results[i]["out"].T
    return out


if __name__ == "__main__":
    if "--build" in sys.argv:
        cen12 = np.linspace(-3.0, 3.0, NB1)
        nc = build_bass(cen12, NBLK)
        print("build OK, instructions:",
              sum(len(b.instructions) for b in nc.main_func.blocks))
        sys.exit(0)
    xs = np.random.randn(BATCH, SEQ).astype(np.float32)
    c1 = (np.random.randn(HID, IN_DIM, NB) * 0.05).astype(np.float32)
    c2 = (np.random.randn(OUT_DIM, HID, NB) * 0.05).astype(np.float32)
    cen = np.linspace(-3, 3, NB).astype(np.float32)
    print(kernel(xs, c1, c2, cen)[:2])


# revision 18
# speedup vs baseline: 1.0519x; 1.0519x over previous
"""BayesianKAN ECG kernel for Trainium2 (8 NeuronCores, data-parallel batch shard).

Pipeline per 1024-row pair of 512-row blocks:
  x[128,2,1000] --gp add + DVE reduce--> pooled[128,8,100] --bn_stats-->
  batched Newton rsqrt --> normalize (fp32) --PE transpose--> xnT[100,1024]
  b1[100,12,1024] bf16 (reduced 12-Gaussian basis, host-refit weights):
    3 ACT anchor slices (Square+Exp) + ru/rd exps + 2 batched DVE ring muls
  --12 PE matmuls bf16--> h[64,1024] --Tanh--> hb bf16
  --8 DMA-transposes--> layout A --stats+Newton+normalize (fp32)--> hn
  --8 PE transposes--> b2s[128,1024] fp32 (DMA replicate)
  b2[128,6,1024] bf16: 1 ACT anchor chunk + ru2/rd2 + 4 DVE ring muls
  --12 PE matmuls--> out[5,1024] fp32 --> DRAM
Notes:
  * mean/std scale-invariant so pooling /10 skipped.
  * ddof=1 via biased-var scale n/(n-1) before Newton rsqrt.
  * the 16-Gaussian basis is least-squares projected onto 12 wider-spaced
    Gaussians on the host (weighted by the N(0,1) input density); residual
    ~1e-3 relative.
  * basis slices away from anchors use B_n = B_a * r^(n-a) with
    r = exp(2*A*delta*x); the constant exp(-A*(c_n^2-c_a^2)) is folded into
    the matmul weights on the host.
  * PSUM->SBUF copies ride the Scalar engine (AF.Copy) to offload DVE.
"""

import os
import sys
from contextlib import ExitStack

import numpy as np
import ml_dtypes

sys.path.insert(0, "/opt/trn_rl_repo")

import concourse.bass as bass
import concourse.tile as tile
from concourse import mybir
from concourse.bass_utils import run_bass_kernel_spmd

from concourse import bass2jax as _b2j

_orig_hook = _b2j.neuronx_cc_hook


def _dbg_hook(*a, **k):
    try:
        return _orig_hook(*a, **k)
    except BaseException:
        import traceback
        with open("/tmp/hook_err.txt", "w") as f:
            traceback.print_exc(file=f)
        raise


_b2j.neuronx_cc_hook = _dbg_hook


MAX_WAITS = 1


def _split_sync_waits(nc, limit=MAX_WAITS):
    """Walrus in this env rejects instructions with more than ~2 sync waits.

    Move excess waits onto same-engine NOPs inserted right before the
    offending instruction (in-order engines make this equivalent).
    """
    n_split = 0
    for block in nc.main_func.blocks:
        new_insts = []
        for inst in block.instructions:
            si = inst.sync_info
            waits = list(si.on_wait) if si is not None else []
            if len(waits) > limit:
                extra, keep = waits[:-limit], waits[-limit:]
                for k in range(0, len(extra), limit):
                    nop = mybir.InstNoOp(
                        name=f"{inst.name}-ws{k}",
                        sync_info=mybir.SyncInfo(
                            on_wait=extra[k : k + limit], on_update=[]
                        ),
                        bass_nofuse=True,
                        engine=inst.engine,
                    )
                    nc.register_instruction(nop, overwrite=True)
                    new_insts.append(nop)
                    n_split += 1
                si.on_wait = keep
                inst.sync_info = si
            new_insts.append(inst)
        block.instructions[:] = new_insts
    return n_split


BATCH = 65536
SEQ = 1000
IN_DIM = 100
POOLW = 10
HID = 64
OUT_DIM = 5
NB = 16
NCORES = 8
ROWS = BATCH // NCORES          # 8192 rows per core
TILE_P = 128
BLK = 512                       # rows per block
NSUB = BLK // TILE_P            # 4 sub-tiles per block
NBLK = ROWS // BLK              # 16 blocks per core
PBLK = 2 * BLK                  # 1024 rows per pair

NB1 = 12                        # reduced basis size
NCH2 = NB1 // 2                 # b2 chunks
B1_ANCH = (1, 5, 9)             # direct b1 slices
B2_ANCH = 2                     # direct b2 chunk (n = 4, 5)

LAST_RESULTS = None


def _ensure_ntff_hook():
    """Synthesize antenv.axon_hooks (absent in this image) so trace=True works."""
    import types

    if "antenv.axon_hooks" in sys.modules:
        return
    mod = types.ModuleType("antenv.axon_hooks")
    mod._hook = None

    def set_axon_ntff_profile_hook(h):
        mod._hook = h

    def get_axon_ntff_profile_hook():
        return mod._hook

    mod.set_axon_ntff_profile_hook = set_axon_ntff_profile_hook
    mod.get_axon_ntff_profile_hook = get_axon_ntff_profile_hook
    sys.modules["antenv.axon_hooks"] = mod
    import antenv

    antenv.axon_hooks = mod
    try:
        from trn_agent_boot.trn_boot import _ntff_profile_via_ctypes

        hook = _ntff_profile_via_ctypes("/opt/axon/libaxon_pjrt.so")
        if hook is not None:
            set_axon_ntff_profile_hook(hook)
    except Exception as e:
        print("ntff hook setup failed:", e)


RBF_A = np.float32(0.5 / 0.36)  # exp(-A * d^2)
MAGIC = 0x5F3759DF

F32 = mybir.dt.float32
BF16 = mybir.dt.bfloat16
I32 = mybir.dt.int32
AF = mybir.ActivationFunctionType
ALU = mybir.AluOpType
AX = mybir.AxisListType


def _newton_rsqrt(nc, pool, var_ap, out_ap, n_par, n_free, ddof_scale):
    """out = rsqrt(var * ddof_scale) elementwise on a [n_par, n_free] AP.

    var_ap is strided fp32; Newton w/ fast-inverse-sqrt seed, 2 iterations.
    """
    v = pool.tile([n_par, n_free], F32, tag="nw_v")
    y = pool.tile([n_par, n_free], F32, tag="nw_y")
    t = pool.tile([n_par, n_free], F32, tag="nw_t")
    nc.vector.tensor_scalar(v, var_ap, float(ddof_scale), None, ALU.mult)
    nc.vector.tensor_scalar(
        y.bitcast(I32), v.bitcast(I32), 1, None, ALU.logical_shift_right
    )
    nc.vector.tensor_scalar(
        y.bitcast(I32), y.bitcast(I32), -1, MAGIC, ALU.mult, ALU.add
    )
    nc.vector.tensor_mul(t, y, y)
    nc.vector.tensor_mul(t, t, v)
    nc.vector.tensor_scalar(t, t, -0.5, 1.5, ALU.mult, ALU.add)
    nc.vector.tensor_mul(out_ap, y, t)


def build_bass(cen12, nblk=NBLK):
    rows = nblk * BLK
    npairs = nblk // 2
    assert nblk % 2 == 0
    nc = bass.Bass()

    x_in = nc.declare_dram_parameter("x", [rows, SEQ], F32, isOutput=False)
    w1_in = nc.declare_dram_parameter("w1", [IN_DIM, NB1 * HID], BF16, isOutput=False)
    w2_in = nc.declare_dram_parameter(
        "w2", [TILE_P, NCH2 * OUT_DIM], BF16, isOutput=False
    )
    cb1_in = nc.declare_dram_parameter("cb1", [IN_DIM, 3], F32, isOutput=False)
    cb2a_in = nc.declare_dram_parameter("cb2a", [TILE_P, 1], F32, isOutput=False)
    ident_in = nc.declare_dram_parameter("ident", [TILE_P, TILE_P], F32, isOutput=False)
    identb_in = nc.declare_dram_parameter("identb", [HID, HID], BF16, isOutput=False)
    out_ext = nc.declare_dram_parameter("out", [OUT_DIM, rows], F32, isOutput=True)

    cvals = [float(c) for c in cen12]
    delta = cvals[1] - cvals[0]
    A = float(RBF_A)
    RU_S = 2.0 * A * delta          # ru = exp(+RU_S * x), rd = exp(-RU_S * x)
    RU2_S = 4.0 * A * delta         # chunk step is 2*delta

    with ExitStack() as ctx:
        tc = ctx.enter_context(tile.TileContext(nc))
        singles = ctx.enter_context(tc.tile_pool(name="singles", bufs=1))
        xin_p = ctx.enter_context(tc.tile_pool(name="xin", bufs=6))
        xh_p = ctx.enter_context(tc.tile_pool(name="xh", bufs=4))
        xp_p = ctx.enter_context(tc.tile_pool(name="xp", bufs=3))
        small_p = ctx.enter_context(tc.tile_pool(name="small", bufs=4))
        nw_p = ctx.enter_context(tc.tile_pool(name="newton", bufs=2))
        xn_p = ctx.enter_context(tc.tile_pool(name="xn", bufs=2))
        xnt_p = ctx.enter_context(tc.tile_pool(name="xnt", bufs=2))
        b1sq_p = ctx.enter_context(tc.tile_pool(name="b1sq", bufs=1))
        b1_p = ctx.enter_context(tc.tile_pool(name="b1", bufs=2))
        rdru_p = ctx.enter_context(tc.tile_pool(name="rdru", bufs=2))
        hb_p = ctx.enter_context(tc.tile_pool(name="hb", bufs=2))
        ha_p = ctx.enter_context(tc.tile_pool(name="ha", bufs=2))
        hn_p = ctx.enter_context(tc.tile_pool(name="hn", bufs=2))
        b2s_p = ctx.enter_context(tc.tile_pool(name="b2s", bufs=2))
        b2sq_p = ctx.enter_context(tc.tile_pool(name="b2sq", bufs=1))
        rdru2_p = ctx.enter_context(tc.tile_pool(name="rdru2", bufs=1))
        b2_p = ctx.enter_context(tc.tile_pool(name="b2", bufs=2))
        outs_p = ctx.enter_context(tc.tile_pool(name="outs", bufs=3))
        ps_t = ctx.enter_context(tc.tile_pool(name="ps_t", bufs=2, space="PSUM"))
        ps_h = ctx.enter_context(tc.tile_pool(name="ps_h", bufs=1, space="PSUM"))
        ps_x2 = ctx.enter_context(tc.tile_pool(name="ps_x2", bufs=1, space="PSUM"))
        ps_o = ctx.enter_context(tc.tile_pool(name="ps_o", bufs=1, space="PSUM"))
        ps_a = ctx.enter_context(tc.tile_pool(name="ps_a", bufs=1, space="PSUM"))

        ident = singles.tile([TILE_P, TILE_P], F32)
        nc.sync.dma_start(out=ident, in_=ident_in[:, :])
        identb = singles.tile([HID, HID], BF16)
        nc.sync.dma_start(out=identb, in_=identb_in[:, :])
        w1 = singles.tile([IN_DIM, NB1, HID], BF16)
        nc.sync.dma_start(out=w1, in_=w1_in[:, :].rearrange("i (n o) -> i n o", n=NB1))
        w2 = singles.tile([TILE_P, NCH2, OUT_DIM], BF16)
        nc.sync.dma_start(out=w2, in_=w2_in[:, :].rearrange("p (c o) -> p c o", c=NCH2))
        cb1 = singles.tile([IN_DIM, 3], F32)
        nc.sync.dma_start(out=cb1, in_=cb1_in[:, :])
        cb2a = singles.tile([TILE_P, 1], F32)
        nc.sync.dma_start(out=cb2a, in_=cb2a_in[:, :])

        # rows index = ((b*2 + s2)*2 + k)*128 + p
        x_t = x_in[:, :].rearrange(
            "(nb s2 k p) s -> nb s2 p k s", nb=nblk, s2=2, k=2
        )

        pairs = {}

        def stage_a(b):
            """DMA + pool + stats for block b into the pair's tiles."""
            pr = b // 2
            if b % 2 == 0:
                pairs[pr] = {
                    "xp": xp_p.tile([TILE_P, 8, IN_DIM], F32, tag="xp", name="xp"),
                    "mv1": small_p.tile([TILE_P, 2, 8], F32, tag="mv1", name="mv1"),
                }
            d = pairs[pr]
            base = (b % 2) * NSUB
            for s2 in range(2):
                x2 = xin_p.tile([TILE_P, 2, SEQ], F32, tag="x")
                nc.sync.dma_start(out=x2, in_=x_t[b, s2])
                xf = x2.rearrange("p k (i w) -> p k i w", w=POOLW)
                xh = xh_p.tile([TILE_P, 2, IN_DIM, POOLW // 2], F32, tag="xh")
                nc.gpsimd.tensor_tensor(
                    xh, xf[:, :, :, 0 : POOLW // 2], xf[:, :, :, POOLW // 2 :],
                    ALU.add,
                )
                sg = base + 2 * s2
                nc.vector.tensor_reduce(
                    d["xp"][:, sg : sg + 2, :], xh, AX.X, ALU.add
                )
                for k in range(2):
                    st6 = small_p.tile([TILE_P, 6], F32, tag="st6")
                    nc.vector.bn_stats(st6, d["xp"][:, sg + k, :])
                    nc.vector.bn_aggr(d["mv1"][:, :, sg + k : sg + k + 1], st6)

        def stage_t(pr):
            """Batched Newton + normalize + PE transpose -> xnt fp32."""
            d = pairs[pr]
            r1 = small_p.tile([TILE_P, 8], F32, tag="r1")
            _newton_rsqrt(
                nc, nw_p, d["mv1"][:, 1, :], r1, TILE_P, 8, IN_DIM / (IN_DIM - 1)
            )
            xn = xn_p.tile([TILE_P, 8, IN_DIM], F32, tag="xn")
            pss = []
            for half in range(2):
                xnt_ps = ps_t.tile([IN_DIM, BLK], F32, tag="xnt_ps")
                for s in range(NSUB):
                    sg = half * NSUB + s
                    nc.vector.tensor_scalar(
                        xn[:, sg, :], d["xp"][:, sg, :],
                        d["mv1"][:, 0, sg : sg + 1], r1[:, sg : sg + 1],
                        ALU.subtract, ALU.mult,
                    )
                    nc.tensor.transpose(
                        xnt_ps[:, s * TILE_P : (s + 1) * TILE_P],
                        xn[:, sg, :], ident,
                    )
                pss.append(xnt_ps)
            del pairs[pr]
            return pss

        def stage_l1_basis(pr, pss):
            """b1 basis: ACT anchors/ratios + DVE ring muls (no matmuls)."""
            b1sq = b1sq_p.tile([IN_DIM, 3, PBLK], F32, tag="b1sq")
            b1 = b1_p.tile([IN_DIM, NB1, PBLK], BF16, tag="b1")
            rdru = rdru_p.tile([IN_DIM, 2, PBLK], BF16, tag="rdru")
            for half in range(2):
                sl = slice(half * BLK, (half + 1) * BLK)
                nc.scalar.activation(rdru[:, 0, sl], pss[half], AF.Exp, scale=-RU_S)
                nc.scalar.activation(rdru[:, 1, sl], pss[half], AF.Exp, scale=RU_S)
                for j, n in enumerate(B1_ANCH):
                    nc.scalar.activation(
                        b1sq[:, j, sl], pss[half], AF.Square,
                        bias=cb1[:, j : j + 1],
                    )
            b1g4 = b1.rearrange("p (g a) k -> p g a k", a=4)
            nc.scalar.activation(
                b1g4[:, :, 1, :], b1sq, AF.Exp, scale=float(-RBF_A)
            )
            # ring1: slices 4g (rd) and 4g+2 (ru) from anchor 4g+1, one op
            b1r = b1.rearrange("p (g j b) k -> p g j b k", j=2, b=2)
            dst = b1r[:, :, :, 0, :]
            src = b1g4[:, :, 1, :].unsqueeze(2).broadcast_to((IN_DIM, 3, 2, PBLK))
            mlt = rdru.unsqueeze(1).broadcast_to((IN_DIM, 3, 2, PBLK))
            nc.vector.tensor_tensor(dst, src, mlt, ALU.mult)
            # ring2: slices {3,7,11} = {2,6,10} * ru, one op
            nc.vector.tensor_tensor(
                b1g4[:, :, 3, :], b1g4[:, :, 2, :],
                rdru[:, 1, :].unsqueeze(1).broadcast_to((IN_DIM, 3, PBLK)),
                ALU.mult,
            )
            return b1

        def stage_l1_mm(pr, b1):
            """mm1 + tanh -> hb bf16."""
            h_ps = ps_h.tile([HID, PBLK], F32, tag="h_ps")
            mm_order = list(B1_ANCH) + [0, 2, 4, 6, 8, 10, 3, 7, 11]
            for mi, n in enumerate(mm_order):
                for hf in range(2):
                    nc.tensor.matmul(
                        h_ps[:, hf * BLK : (hf + 1) * BLK],
                        w1[:, n, :],
                        b1[:, n, hf * BLK : (hf + 1) * BLK],
                        start=(mi == 0), stop=(mi == len(mm_order) - 1),
                    )
            hb = hb_p.tile([HID, PBLK], BF16, tag="hb")
            nc.scalar.activation(hb, h_ps, AF.Tanh)
            return hb

        def stage_l2_pre_a(pr, hb):
            """norm2 front: PE transpose to layout A (PSUM), stats, Newton,
            normalize (reads PSUM directly)."""
            ha_ps = ps_a.tile([TILE_P, 8 * HID], BF16, tag="ha_ps")
            for c in range(8):
                nc.tensor.transpose(
                    ha_ps[:, c * HID : (c + 1) * HID],
                    hb[:, c * TILE_P : (c + 1) * TILE_P], identb,
                )
            mv2 = small_p.tile([TILE_P, 2, 8], F32, tag="mv2")
            r2 = small_p.tile([TILE_P, 8], F32, tag="r2")
            for c in range(8):
                st6b = small_p.tile([TILE_P, 6], F32, tag="st6b")
                nc.vector.bn_stats(st6b, ha_ps[:, c * HID : (c + 1) * HID])
                nc.vector.bn_aggr(mv2[:, :, c : c + 1], st6b)
            _newton_rsqrt(
                nc, nw_p, mv2[:, 1, :], r2, TILE_P, 8, HID / (HID - 1)
            )
            hn = hn_p.tile([TILE_P, 8, HID], F32, tag="hn")
            for c in range(8):
                nc.vector.tensor_scalar(
                    hn[:, c, :], ha_ps[:, c * HID : (c + 1) * HID],
                    mv2[:, 0, c : c + 1], r2[:, c : c + 1],
                    ALU.subtract, ALU.mult,
                )
            return hn

        def stage_l2_pre_b(pr, hn):
            """norm2 back: PE transpose to layout B, replicate -> b2s fp32."""
            xn2_ps = ps_x2.tile([HID, PBLK], F32, tag="xn2_ps")
            for c in range(8):
                nc.tensor.transpose(
                    xn2_ps[:, c * TILE_P : (c + 1) * TILE_P], hn[:, c, :], ident
                )
            b2s = b2s_p.tile([TILE_P, PBLK], F32, tag="b2s")
            nc.scalar.activation(b2s[:HID, :], xn2_ps, AF.Copy)
            nc.scalar.dma_start(out=b2s[HID:, :], in_=b2s[:HID, :])
            return b2s

        def stage_l2_basis(pr, b2s):
            """b2 basis: ACT anchor chunk + ratios + DVE ring muls."""
            b2sq = b2sq_p.tile([TILE_P, PBLK], F32, tag="b2sq")
            b2 = b2_p.tile([TILE_P, NCH2, PBLK], BF16, tag="b2")
            rdru2 = rdru2_p.tile([TILE_P, 2, PBLK], BF16, tag="rdru2")
            nc.scalar.activation(rdru2[:, 0, :], b2s, AF.Exp, scale=-RU2_S)
            nc.scalar.activation(rdru2[:, 1, :], b2s, AF.Exp, scale=RU2_S)
            nc.scalar.activation(b2sq, b2s, AF.Square, bias=cb2a[:, 0:1])
            nc.scalar.activation(
                b2[:, B2_ANCH, :], b2sq, AF.Exp, scale=float(-RBF_A)
            )
            # ring1: chunks {1,3} from anchor 2
            b2r2 = b2.rearrange("p (a b) k -> p a b k", b=2)
            dst1 = b2r2[:, 0:2, 1, :]
            src1 = b2[:, B2_ANCH, :].unsqueeze(1).broadcast_to((TILE_P, 2, PBLK))
            nc.vector.tensor_tensor(dst1, src1, rdru2, ALU.mult)
            # chunk 0 from 1 (rd2); chunk 4 from 3 (ru2); chunk 5 from 4 (ru2)
            nc.gpsimd.tensor_tensor(
                b2[:, 0, :], b2[:, 1, :], rdru2[:, 0, :], ALU.mult
            )
            nc.vector.tensor_tensor(
                b2[:, 4, :], b2[:, 3, :], rdru2[:, 1, :], ALU.mult
            )
            nc.vector.tensor_tensor(
                b2[:, 5, :], b2[:, 4, :], rdru2[:, 1, :], ALU.mult
            )
            return b2

        def stage_l2_mm(pr, b2):
            """mm2 + out copies + out DMA."""
            mm2_order = [2, 1, 3, 0, 4, 5]
            for hf in range(2):
                o_ps = ps_o.tile([OUT_DIM, BLK], F32, tag="o_ps")
                for ci, c in enumerate(mm2_order):
                    nc.tensor.matmul(
                        o_ps,
                        w2[:, c, :],
                        b2[:, c, hf * BLK : (hf + 1) * BLK],
                        start=(ci == 0), stop=(ci == len(mm2_order) - 1),
                    )
                out_sb = outs_p.tile([OUT_DIM, BLK], F32, tag="out_sb")
                nc.scalar.activation(out_sb, o_ps, AF.Copy)
                bb = 2 * pr + hf
                nc.scalar.dma_start(
                    out=out_ext[:, bb * BLK : (bb + 1) * BLK], in_=out_sb
                )

        # software pipeline (lagged): iteration p emits
        #   T(p), L1basis(p), L2pre_a(p-1), L1mm(p), L2pre_b(p-1),
        #   L2basis(p-2), L2mm(p-3), stage_a(pair p+2 blocks)
        # so each engine FIFO always holds ready work from several pairs.
        for b in (0, 1, 2, 3):
            if b < nblk:
                stage_a(b)
        hbs, hns, b2ss, b2sb = {}, {}, {}, {}
        for p in range(npairs + 3):
            if p < npairs:
                pss = stage_t(p)
                b1 = stage_l1_basis(p, pss)
            if p - 1 in hbs:
                hns[p - 1] = stage_l2_pre_a(p - 1, hbs.pop(p - 1))
            if p < npairs:
                hbs[p] = stage_l1_mm(p, b1)
            if p - 1 in hns:
                b2ss[p - 1] = stage_l2_pre_b(p - 1, hns.pop(p - 1))
            if p - 2 in b2ss:
                b2sb[p - 2] = stage_l2_basis(p - 2, b2ss.pop(p - 2))
            if p - 3 in b2sb:
                stage_l2_mm(p - 3, b2sb.pop(p - 3))
            for b in (2 * p + 4, 2 * p + 5):
                if b < nblk:
                    stage_a(b)

    _split_sync_waits(nc)
    return nc


def _reduce_basis_matrix(cen16, cen12):
    """Least-squares projection of the 16-Gaussian basis onto 12 Gaussians."""
    A = float(RBF_A)
    xg = np.linspace(-8, 8, 4001)
    wd = np.sqrt(np.exp(-0.5 * xg**2) + 1e-4)
    Phi = np.exp(-A * (xg[:, None] - cen16[None, :]) ** 2)
    Psi = np.exp(-A * (xg[:, None] - cen12[None, :]) ** 2)
    M, *_ = np.linalg.lstsq(Psi * wd[:, None], Phi * wd[:, None], rcond=None)
    return M  # [12, 16]


def _host_consts(c1_mu, c2_mu, centers):
    A = float(RBF_A)
    cen16 = np.asarray(centers, np.float64)
    cen12 = np.linspace(cen16[0], cen16[-1], NB1)
    M = _reduce_basis_matrix(cen16, cen12)
    c1m = np.tensordot(c1_mu.astype(np.float64), M, axes=([2], [1]))  # [o,i,12]
    c2m = np.tensordot(c2_mu.astype(np.float64), M, axes=([2], [1]))  # [o,h,12]

    # b1 chain anchor: every slice chains to 4*(n//4)+1
    w1f = np.transpose(c1m, (1, 2, 0)).copy()  # [i, n, o]
    for n in range(NB1):
        a = 4 * (n // 4) + 1
        if a != n:
            w1f[:, n, :] *= np.exp(-A * (cen12[n] ** 2 - cen12[a] ** 2))
    w1 = w1f.reshape(IN_DIM, NB1 * HID).astype(ml_dtypes.bfloat16)

    # b2: partition p of chunk c holds i = p % 64, n = 2c + p // 64
    w2 = np.zeros((TILE_P, NCH2, OUT_DIM), np.float64)
    cb2a = np.zeros((TILE_P, 1), np.float32)
    for p in range(TILE_P):
        i = p % HID
        na = 2 * B2_ANCH + p // HID
        cb2a[p, 0] = -cen12[na]
        for c in range(NCH2):
            n = 2 * c + p // HID
            k = np.exp(-A * (cen12[n] ** 2 - cen12[na] ** 2)) if c != B2_ANCH else 1.0
            w2[p, c, :] = c2m[:, i, n] * k
    w2 = w2.reshape(TILE_P, NCH2 * OUT_DIM).astype(ml_dtypes.bfloat16)
    cb1 = np.tile(
        -cen12[np.array(B1_ANCH)].astype(np.float32)[None, :], (IN_DIM, 1)
    ).astype(np.float32)
    ident = np.eye(TILE_P, dtype=np.float32)
    identb = np.eye(HID, dtype=ml_dtypes.bfloat16)
    return w1, w2, cb1, cb2a, ident, identb, cen12


def kernel(x, c1_mu, c2_mu, centers):
    x = np.asarray(x, np.float32)
    batch = x.shape[0]
    rows = batch // NCORES
    nblk = rows // BLK
    assert rows % BLK == 0 and nblk % 2 == 0
    c1_mu = np.asarray(c1_mu, np.float32)
    c2_mu = np.asarray(c2_mu, np.float32)
    centers = np.asarray(centers, np.float32)

    w1, w2, cb1, cb2a, ident, identb, cen12 = _host_consts(c1_mu, c2_mu, centers)
    nc = build_bass(cen12, nblk)

    in_maps = []
    for i in range(NCORES):
        in_maps.append(
            {
                "x": np.ascontiguousarray(x[i * rows : (i + 1) * rows]),
                "w1": w1,
                "w2": w2,
                "cb1": cb1,
                "cb2a": cb2a,
                "ident": ident,
                "identb": identb,
            }
        )
    trace = bool(int(os.environ.get("BASS_KERNEL_TRACE", "0")))
    if trace:
        sys.path.insert(0, "/root/.axon_site")
        _ensure_ntff_hook()
    res = run_bass_kernel_spmd(
        nc, in_maps, list(range(NCORES)), trace=trace
    )
    global LAST_RESULTS
    LAST_RESULTS = res
    out = np.empty((batch, OUT_DIM), np.float32)
    for i in range(NCORES):
        out[i * rows : (i + 1) * rows] = res.results[i]["out"].T
    return out


if __name__ == "__main__":
    if "--build" in sys.argv:
        cen12 = np.linspace(-3.0, 3.0, NB1)
        nc = build_bass(cen12, NBLK)
        print("build OK, instructions:",
              sum(len(b.instructions) for b in nc.main_func.blocks))
        sys.exit(0)
    xs = np.random.randn(BATCH, SEQ).astype(np.float32)
    c1 = (np.random.randn(HID, IN_DIM, NB) * 0.05).astype(np.float32)
    c2 = (np.random.randn(OUT_DIM, HID, NB) * 0.05).astype(np.float32)
    cen = np.linspace(-3, 3, NB).astype(np.float32)
    print(kernel(xs, c1, c2, cen)[:2])
